# revision 20
# baseline (speedup 1.0000x reference)
"""DGN network (encoder MLP -> 2x TransformerConv -> per-agent readout) on TRN2.

Strategy
--------
Data-parallel over the 32 independent subgraphs: 4 graphs per NeuronCore.
The final output reads conv2 only at each graph's agent node, so per graph
only the agent's 1-hop set D1 = {agent} u N(agent) (conv1 dst nodes, needed
for conv2's q/k/v) and the 2-hop set S1 = D1 u N(D1) (encoder + conv1 src
nodes) participate.  The host builds the index lists and exact additive
adjacency masks (log-multiplicity, built from edge_index as given, so
duplicate edges / self loops / isolated agents are handled exactly); the
device gathers node features with indirect DMA and runs small dense masked
attention on the gathered sets.  All matmuls, softmaxes and aggregations
run on hardware in fp32.

Device layout (default P_D=32, P_S=128, 4 graphs/core):
  - encoder / q / k in transposed layout [feat, node]; biases folded via a
    ones row in the augmented weights or scalar.activation per-partition.
  - attention is computed entirely in source-major (transposed) layout:
    scores^T [src, dst], e = exp((s + 8*mask)/8) without max subtraction
    (scores are O(0.1); absent edges get -8e30 -> exp == 0 exactly), the
    softmax denominator comes free from a ones column packed into the V
    weights, and the normalization 1/z is applied as a rank-1 outer
    product (PE) times the aggregated output -- no PE transposes of the
    attention matrix, no row-major score tiles.
"""

from contextlib import ExitStack

import numpy as np

BS, N, D = 32, 1000, 6
HID, H = 64, 4
OUT = 5
N_CORES = 8
GPC = BS // N_CORES  # graphs per core
NEG8 = -8.0e30       # additive mask, pre-multiplied by sqrt(HID)=8
EPS = 1.0e-30

_COMPILE_CACHE: dict = {}


# ----------------------------------------------------------------------------
# Host-side preprocessing: index sets + masks (pure index manipulation).
# ----------------------------------------------------------------------------

def _preprocess(obs: np.ndarray, edge_index: np.ndarray):
    obs = np.asarray(obs, dtype=np.float32)
    ei = np.asarray(edge_index)
    src = ei[0].astype(np.int64)
    dst = ei[1].astype(np.int64)

    node_feats = np.ascontiguousarray(obs[:, : N * 8].reshape(BS * N, 8)[:, 2:8])
    agent = np.clip(obs[:, -1], 0, N - 1).astype(np.int32)
    agent_glob = (np.arange(BS, dtype=np.int64) * N) + agent

    # CSR of in-edges (grouped by dst), multiplicity preserved
    order = np.argsort(dst, kind="stable")
    sdst = dst[order]
    ssrc = src[order]
    bounds = np.searchsorted(sdst, np.arange(BS * N + 1))

    def in_srcs(v):
        return ssrc[bounds[v] : bounds[v + 1]]

    D1_list, S1_list = [], []
    for b in range(BS):
        a = int(agent_glob[b])
        nbr = in_srcs(a)
        others = np.unique(nbr)
        others = others[others != a]
        D1 = np.concatenate([[a], others]).astype(np.int64)
        srcs_all = np.unique(np.concatenate([in_srcs(int(u)) for u in D1]))
        extra = np.setdiff1d(srcs_all, D1)
        S1 = np.concatenate([D1, extra])
        D1_list.append(D1)
        S1_list.append(S1)

    max_d1 = max(len(x) for x in D1_list)
    max_s1 = max(len(x) for x in S1_list)
    P_D = 32 * ((max_d1 + 31) // 32)
    P_S = 32 * ((max_s1 + 31) // 32)
    assert P_D <= 32, f"agent degree too large for this layout: |D1|={max_d1}"
    assert P_S <= 512, f"2-hop set too large: |S1|={max_s1}"
    SC = (P_S + 127) // 128    # source chunks per graph (last may be short)

    # Per-graph gather indices (padded with 0 -> harmless real data, masked)
    gidx = np.zeros((BS, P_S), np.int32)
    # conv1 mask, transposed and pre-scaled by 8: [src position, dst position]
    m1t8 = np.full((BS, P_S, P_D), NEG8, np.float32)
    # conv2 mask over D1 source positions
    m2t8 = np.full((BS, P_D), NEG8, np.float32)

    for b in range(BS):
        D1, S1 = D1_list[b], S1_list[b]
        gidx[b, : len(S1)] = S1
        pos = {int(v): i for i, v in enumerate(S1)}
        for up, u in enumerate(D1):
            s_of_u = in_srcs(int(u))
            if len(s_of_u) == 0:
                continue
            vals, cnts = np.unique(s_of_u, return_counts=True)
            for v, c in zip(vals, cnts):
                m1t8[b, pos[int(v)], up] = 8.0 * np.log(np.float32(c)) if c > 1 else 0.0
        a_srcs = in_srcs(int(D1[0]))
        if len(a_srcs):
            vals, cnts = np.unique(a_srcs, return_counts=True)
            for v, c in zip(vals, cnts):
                sp = pos[int(v)]
                assert sp < len(D1)
                m2t8[b, sp] = 8.0 * np.log(np.float32(c)) if c > 1 else 0.0

    return dict(
        node_feats=node_feats,
        gidx=gidx,
        m1t8=m1t8,
        m2t8=m2t8,
        P_D=P_D,
        P_S=P_S,
        SC=SC,
    )


def _pack_v_weights(wv, bv, n_in):
    """[n_in+1, 65*H] tile: head h -> cols [65h:65h+64] = Wv head block (with
    bias row at n_in); col 65h+64 = basis vector selecting the ones row, so
    the AV matmul's M=65 stationary also produces the softmax denominator."""
    p = np.zeros((n_in + 1, 65 * H), np.float32)
    for h in range(H):
        p[:n_in, 65 * h : 65 * h + HID] = wv[:, HID * h : HID * (h + 1)]
        p[n_in, 65 * h : 65 * h + HID] = bv[HID * h : HID * (h + 1)]
        p[n_in, 65 * h + HID] = 1.0
    return p


def _chunks(P_S):
    out = []
    while P_S > 0:
        out.append(min(128, P_S))
        P_S -= out[-1]
    return out


def _pack_layout(SC):
    """Column layout of the single consolidated [128, X] weight/mask pack."""
    specs = [
        ("w1", D, HID),
        ("w2", HID, HID),
        ("b12", HID, 2),
        ("wq1a", HID + 1, H * HID),
        ("wk1a", HID + 1, H * HID),
        ("wv1p", HID + 1, 65 * H),
        ("q2_k0", 128, H * HID), ("q2_k1", 128, H * HID), ("q2_kb", 1, H * HID),
        ("k2_k0", 128, H * HID), ("k2_k1", 128, H * HID), ("k2_kb", 1, H * HID),
        ("v2_k0", 128, 65 * H), ("v2_k1", 128, 65 * H), ("v2_kb", 1, 65 * H),
        ("wout0", 128, OUT), ("wout1", 128, OUT), ("woutb", 1, OUT),
        ("gidx", 128, GPC * SC),
    ]
    specs += [(f"m1_{k}", 128, 128) for k in range(SC)]
    specs += [("m2", 128, H * GPC)]
    layout, c = {}, 0
    for name, rows, cols in specs:
        layout[name] = (c, c + cols, rows)
        c += cols
    layout["_total"] = c
    return layout


def _per_core_inputs(pre, weights):
    P_D, P_S, SC = pre["P_D"], pre["P_S"], pre["SC"]
    w = weights
    layout = _pack_layout(SC)
    base = np.zeros((128, layout["_total"]), np.float32)

    def put(name, arr):
        c0, c1, rows = layout[name]
        assert arr.shape == (rows, c1 - c0), (name, arr.shape)
        base[:rows, c0:c1] = arr

    wq2a = np.vstack([w["c2_wq"], w["c2_bq"][None, :]]).astype(np.float32)
    wk2a = np.vstack([w["c2_wk"], w["c2_bk"][None, :]]).astype(np.float32)
    wv2p = _pack_v_weights(
        np.asarray(w["c2_wv"], np.float32), np.asarray(w["c2_bv"], np.float32), H * HID
    )
    wouta = np.vstack([w["out_w"], w["out_b"][None, :]]).astype(np.float32)

    put("w1", np.asarray(w["enc_w1"], np.float32))
    put("w2", np.asarray(w["enc_w2"], np.float32))
    put("b12", np.stack([w["enc_b1"], w["enc_b2"]], axis=1).astype(np.float32))
    put("wq1a", np.vstack([w["c1_wq"], w["c1_bq"][None, :]]).astype(np.float32))
    put("wk1a", np.vstack([w["c1_wk"], w["c1_bk"][None, :]]).astype(np.float32))
    put("wv1p", _pack_v_weights(
        np.asarray(w["c1_wv"], np.float32), np.asarray(w["c1_bv"], np.float32), HID
    ))
    for nm, arr in (("q2", wq2a), ("k2", wk2a), ("v2", wv2p)):
        put(f"{nm}_k0", arr[0:128])
        put(f"{nm}_k1", arr[128:256])
        put(f"{nm}_kb", arr[256:257])
    put("wout0", wouta[0:128])
    put("wout1", wouta[128:256])
    put("woutb", wouta[256:257])

    in_maps = []
    for c in range(N_CORES):
        pack = base.copy()

        def putc(name, arr):
            c0, c1, rows = layout[name]
            assert arr.shape == (rows, c1 - c0), (name, arr.shape)
            pack[:rows, c0:c1] = arr

        gs = slice(c * GPC, (c + 1) * GPC)
        chunks = _chunks(P_S)
        gi = np.zeros((128, GPC * SC), np.int32)
        for g in range(GPC):
            o = 0
            for k, ck in enumerate(chunks):
                gi[:ck, g * SC + k] = pre["gidx"][c * GPC + g, o : o + ck]
                o += ck
        putc("gidx", gi.view(np.float32))
        # conv1 mask chunks: row s (source local row in chunk), col 32g+u
        o = 0
        for k, ck in enumerate(chunks):
            m1k = np.full((128, 128), NEG8, np.float32)
            for g in range(GPC):
                mg = pre["m1t8"][c * GPC + g]  # [P_S, P_D]
                m1k[:ck, g * P_D : (g + 1) * P_D] = mg[o : o + ck]
            putc(f"m1_{k}", m1k)
            o += ck
        # conv2 mask [P_D, H*GPC] (rows = D1 source position), col h*GPC+g
        m2 = np.full((128, H * GPC), NEG8, np.float32)
        for g in range(GPC):
            for h in range(H):
                m2[:P_D, h * GPC + g] = pre["m2t8"][c * GPC + g]
        putc("m2", m2)
        in_maps.append({"node_feats": pre["node_feats"], "wpack": pack})
    return in_maps


# ----------------------------------------------------------------------------
# Device program
# ----------------------------------------------------------------------------

def _build_program(P_D, P_S, SC, reps=1, enable_asserts=False, debug_taps=()):
    import concourse.bass as bass
    import concourse.tile as tile
    from concourse import bacc, mybir
    from concourse.masks import make_identity

    f32 = mybir.dt.float32
    i32 = mybir.dt.int32
    AF = mybir.ActivationFunctionType

    assert P_D == 32 and GPC == 4
    UW = GPC * P_D              # packed conv1-dst width = 128
    WT = GPC * P_S              # gathered-node columns

    nc = bacc.Bacc(
        "TRN2",
        target_bir_lowering=False,
        debug=False,
        enable_asserts=enable_asserts,
        num_devices=N_CORES,
    )

    layout = _pack_layout(SC)
    tot_cols = layout["_total"]
    nf = nc.dram_tensor("node_feats", (BS * N, D), f32, kind="ExternalInput").ap()
    wpack = nc.dram_tensor("wpack", (128, tot_cols), f32, kind="ExternalInput").ap()
    out = nc.dram_tensor("out", (GPC, OUT), f32, kind="ExternalOutput").ap()

    tap_outs = {
        nm: nc.dram_tensor(f"tap_{nm}", tuple(shp), f32, kind="ExternalOutput").ap()
        for nm, shp in debug_taps
    }

    def tap(nm, ap):
        if nm in tap_outs:
            nc.sync.dma_start(tap_outs[nm], ap)

    with tile.TileContext(nc) as tc, ExitStack() as ctx:
        cp = ctx.enter_context(tc.tile_pool(name="const", bufs=1))
        wp = ctx.enter_context(tc.tile_pool(name="work", bufs=2))
        pp = ctx.enter_context(tc.tile_pool(name="psum", bufs=1, space="PSUM"))

        def ctile(shape, name, dt=f32):
            return cp.tile(shape, dt, tag=name, name=name)

        # ---- constants / weights: ONE consolidated DMA ----
        ident = ctile([128, 128], "ident")
        make_identity(nc, ident[:])
        ones_row = ctile([1, 512], "ones_row")
        nc.gpsimd.memset(ones_row[:], 1.0)

        wpk = ctile([128, tot_cols], "wpk")
        nc.sync.dma_start(wpk[:], wpack)

        def wsl(name, rows):
            c0, c1, _r = layout[name]
            return wpk[0:rows, c0:c1]

        w1_sb = wsl("w1", D)
        w2_sb = wsl("w2", HID)
        b12_sb = wsl("b12", HID)
        wq1_sb = wsl("wq1a", HID + 1)
        wk1_sb = wsl("wk1a", HID + 1)
        wv1_sb = wsl("wv1p", HID + 1)
        w2ch = {
            nm: (wsl(f"{nm}_k0", 128), wsl(f"{nm}_k1", 128), wsl(f"{nm}_kb", 1))
            for nm in ("q2", "k2", "v2")
        }
        wout0 = wsl("wout0", 128)
        wout1 = wsl("wout1", 128)
        woutb = wsl("woutb", 1)
        gidx_sb = wsl("gidx", 128).bitcast(i32)
        m1_sb = [wsl(f"m1_{k}", 128) for k in range(SC)]
        m2_sb = wsl("m2", 128)

        for r in range(reps):
            def wtile(shape, name, dt=f32):
                return wp.tile(shape, dt, tag=name, name=name)

            def ptile(shape, name, tag="mm", bufs=4):
                return pp.tile(shape, f32, tag=tag, name=name, bufs=bufs)

            # ---- gather + encoder (transposed layout) ----
            chunks = _chunks(P_S)
            off = [sum(chunks[:k]) for k in range(SC)]
            featsT = wtile([D, WT], "featsT")
            for g in range(GPC):
                for k in range(SC):
                    ck = chunks[k]
                    base = g * P_S + off[k]
                    rows = wtile([128, D], "gath")
                    nc.gpsimd.indirect_dma_start(
                        out=rows[0:ck, :],
                        out_offset=None,
                        in_=nf,
                        in_offset=bass.IndirectOffsetOnAxis(
                            ap=gidx_sb[0:ck, g * SC + k : g * SC + k + 1], axis=0
                        ),
                    )
                    tp = ptile([D, 128], "gathT")
                    nc.tensor.transpose(tp[:, 0:ck], rows[0:ck, :], ident[0:ck, 0:ck])
                    nc.vector.tensor_copy(
                        featsT[:, base : base + ck], tp[:, 0:ck]
                    )

            h1T = wtile([HID, WT], "h1T")
            h2T = wtile([HID + 1, WT], "h2T")
            nc.gpsimd.memset(h2T[HID : HID + 1, :], 1.0)
            for k in range(0, WT, 512):
                ke = min(k + 512, WT)
                p1 = ptile([HID, 512], "h1ps")
                nc.tensor.matmul(p1[:, : ke - k], w1_sb[:], featsT[:, k:ke])
                nc.scalar.activation(
                    h1T[:, k:ke], p1[:, : ke - k], AF.Relu, bias=b12_sb[:, 0:1]
                )
                p2 = ptile([HID, 512], "h2ps")
                nc.tensor.matmul(p2[:, : ke - k], w2_sb[:], h1T[:, k:ke])
                nc.scalar.activation(
                    h2T[0:HID, k:ke], p2[:, : ke - k], AF.Relu, bias=b12_sb[:, 1:2]
                )

            tap("h2T", h2T[0:HID, :])
            # D1 (dst) columns of h2T: per graph the first P_D of its block
            h2T_d1 = h2T.rearrange("p (g c) -> p g c", g=GPC)[:, :, 0:P_D]

            # ---- conv1 q (narrow: D1 cols only) / k (transposed) ----
            q1T, k1T = [], []
            for mc in range(2):
                tq = wtile([128, UW], f"q1T_{mc}")
                ps = ptile([128, UW], "q1ps")
                nc.tensor.matmul(ps[:], wq1_sb[:, mc * 128 : (mc + 1) * 128], h2T_d1)
                nc.scalar.copy(tq[:], ps[:])
                q1T.append(tq)
                tk = wtile([128, WT], f"k1T_{mc}")
                for k in range(0, WT, 512):
                    ke = min(k + 512, WT)
                    ps2 = ptile([128, 512], "k1ps")
                    nc.tensor.matmul(
                        ps2[:, : ke - k],
                        wk1_sb[:, mc * 128 : (mc + 1) * 128],
                        h2T[:, k:ke],
                    )
                    nc.scalar.copy(tk[:, k:ke], ps2[:, : ke - k])
                k1T.append(tk)

            # ---- conv1 v (row-major per graph x chunk, 65-col head blocks
            # with the ones column baked in) ----
            v1 = {}
            for g in range(GPC):
                for k in range(SC):
                    ck = chunks[k]
                    base = g * P_S + off[k]
                    ps = ptile([128, 65 * H], "v1ps")
                    nc.tensor.matmul(
                        ps[0:ck, :], h2T[:, base : base + ck], wv1_sb[:]
                    )
                    t = wtile([128, 65 * H], f"v1_{g}_{k}")
                    nc.vector.tensor_copy(t[0:ck, :], ps[0:ck, :])
                    v1[(g, k)] = t

            # ---- conv1 attention, all in source-major layout ----
            # e tiles per (head, chunk): [ck src, 128 (4g x 32 dst)]
            e1 = {}
            for h in range(H):
                mc, hr = divmod(h, 2)
                for k in range(SC):
                    ck = chunks[k]
                    sps = ptile([128, UW], f"s1ps_{h}")
                    for g in range(GPC):
                        base = g * P_S + off[k]
                        nc.tensor.matmul(
                            sps[0:ck, g * P_D : (g + 1) * P_D],
                            k1T[mc][hr * HID : (hr + 1) * HID, base : base + ck],
                            q1T[mc][
                                hr * HID : (hr + 1) * HID, g * P_D : (g + 1) * P_D
                            ],
                        )
                    t = wtile([128, UW], f"e1_{h}_{k}")
                    nc.vector.tensor_add(t[0:ck, :], sps[0:ck, :], m1_sb[k][0:ck, :])
                    nc.scalar.activation(t[0:ck, :], t[0:ck, :], AF.Exp, scale=0.125)
                    e1[(h, k)] = t
            for h in range(H):
                tap(f"e1_{h}", e1[(h, 0)][:])

            # conv1 aggregation, row-major [128 (4g x 32 dst), 65*H]:
            # col 65h+64 is the softmax denominator z for head h
            o1 = ptile([128, 65 * H], "o1", tag="acc", bufs=2)
            for g in range(GPC):
                for h in range(H):
                    for k in range(SC):
                        ck = chunks[k]
                        nc.tensor.matmul(
                            o1[g * P_D : (g + 1) * P_D, 65 * h : 65 * h + 65],
                            e1[(h, k)][0:ck, g * P_D : (g + 1) * P_D],
                            v1[(g, k)][0:ck, 65 * h : 65 * h + 65],
                            start=(k == 0),
                            stop=(k == SC - 1),
                            tile_position=(0, g * P_D),
                        )

            tap("o1", o1[:])
            # normalization is a per-partition (per dst node) activation scale
            z1 = wtile([128, H], "z1")
            nc.vector.tensor_scalar_add(z1[:], o1[:, HID : 65 * H : 65], EPS)
            rz1 = wtile([128, H], "rz1")
            nc.vector.reciprocal(rz1[:], z1[:])
            h1cRM = wtile([128, H * HID], "h1cRM")
            for h in range(H):
                nc.scalar.activation(
                    h1cRM[:, HID * h : HID * (h + 1)],
                    o1[:, 65 * h : 65 * h + HID],
                    AF.Relu,
                    scale=rz1[:, h : h + 1],
                )

            tap("h1cRM", h1cRM[:])
            # transpose h1c to feature-major for the conv2 projections
            h1cT = []
            for mc in range(2):
                tp = ptile([128, 128], "h1cTps")
                nc.tensor.transpose(
                    tp[:], h1cRM[:, 128 * mc : 128 * (mc + 1)], ident[:]
                )
                t = wtile([128, 128], f"h1cT_{mc}")
                nc.vector.tensor_copy(t[:], tp[:])
                h1cT.append(t)

            tap("h1cT_0", h1cT[0][:])
            tap("h1cT_1", h1cT[1][:])
            # ---- conv2 projections (agents / D1 nodes only) ----
            agent_cols_a = h1cT[0][:, 0 : UW : P_D]
            agent_cols_b = h1cT[1][:, 0 : UW : P_D]

            def proj2(nm, rhs_tiles, width, name):
                k0, k1_, kb = w2ch[nm]
                outt = []
                for mc in range(2):
                    ps = ptile([128, width], f"{name}ps_{mc}")
                    nc.tensor.matmul(
                        ps[:], k0[:, mc * 128 : (mc + 1) * 128],
                        rhs_tiles[0], start=True, stop=False,
                    )
                    nc.tensor.matmul(
                        ps[:], k1_[:, mc * 128 : (mc + 1) * 128],
                        rhs_tiles[1], start=False, stop=False,
                    )
                    nc.tensor.matmul(
                        ps[:], kb[:, mc * 128 : (mc + 1) * 128],
                        rhs_tiles[2], start=False, stop=True,
                    )
                    t = wtile([128, width], f"{name}_{mc}")
                    nc.vector.tensor_copy(t[:], ps[:])
                    outt.append(t)
                return outt

            q2T = proj2(
                "q2", [agent_cols_a, agent_cols_b, ones_row[:, 0:GPC]], GPC, "q2T"
            )
            k2T = proj2("k2", [h1cT[0][:], h1cT[1][:], ones_row[:, 0:UW]], UW, "k2T")

            # v2 for all D1 nodes at once, kept packed: [128, 65*H]
            vps = ptile([128, 65 * H], "v2ps")
            vk0, vk1, vkb = w2ch["v2"]
            nc.tensor.matmul(vps[:], h1cT[0][:], vk0[:], start=True, stop=False)
            nc.tensor.matmul(vps[:], h1cT[1][:], vk1[:], start=False, stop=False)
            nc.tensor.matmul(vps[:], ones_row[:, 0:UW], vkb[:], start=False, stop=True)
            v2g = []
            for g in range(GPC):
                t = wtile([P_D, 65 * H], f"v2_{g}")
                nc.vector.tensor_copy(t[:], vps[g * P_D : (g + 1) * P_D, :])
                v2g.append(t)

            # ---- conv2 attention: merged scores [128 (all D1), H*GPC],
            # column h*GPC+g; off-diagonal blocks are masked out ----
            s2ps = ptile([128, H * GPC], "s2ps")
            for h in range(H):
                mc, hr = divmod(h, 2)
                nc.tensor.matmul(
                    s2ps[:, h * GPC : (h + 1) * GPC],
                    k2T[mc][hr * HID : (hr + 1) * HID, :],
                    q2T[mc][hr * HID : (hr + 1) * HID, :],
                )
            e2 = wtile([P_D, H * GPC], "e2")
            for g in range(GPC):
                nc.vector.tensor_add(
                    e2[:, g : H * GPC : GPC],
                    s2ps[g * P_D : (g + 1) * P_D, g : H * GPC : GPC],
                    m2_sb[0:P_D, g : H * GPC : GPC],
                )
            nc.scalar.activation(e2[:], e2[:], AF.Exp, scale=0.125)

            # aggregate + z2: [65, H*GPC], col h*GPC+g
            o2 = ptile([65, H * GPC], "o2", tag="acc", bufs=2)
            for h in range(H):
                for g in range(GPC):
                    cc = h * GPC + g
                    nc.tensor.matmul(
                        o2[:, cc : cc + 1],
                        v2g[g][:, 65 * h : 65 * h + 65],
                        e2[:, cc : cc + 1],
                    )

            tap("e2", e2[:])
            if "o2c" in tap_outs:
                o2c = wtile([65, H * GPC], "o2c")
                nc.vector.tensor_copy(o2c[:], o2[:])
                tap("o2c", o2c[:])
            z2row = wtile([1, H * GPC], "z2row")
            nc.vector.tensor_scalar_add(z2row[:], o2[HID : HID + 1, :], EPS)
            rz2row = wtile([1, H * GPC], "rz2row")
            nc.vector.reciprocal(rz2row[:], z2row[:])
            rz2ps = ptile([HID, H * GPC], "rz2ps")
            for h in range(H):
                nc.tensor.matmul(
                    rz2ps[:, h * GPC : (h + 1) * GPC],
                    ones_row[:, 0:HID],
                    rz2row[:, h * GPC : (h + 1) * GPC],
                    tile_position=(0, 0),
                )
            h2T_f = []
            for mc in range(2):
                t = wtile([128, GPC], f"h2T_{mc}")
                for hh in range(2):
                    h = mc * 2 + hh
                    rl = wtile([HID, GPC], f"relu2_{h}")
                    nc.scalar.activation(
                        rl[:], o2[0:HID, h * GPC : (h + 1) * GPC], AF.Relu
                    )
                    nc.vector.tensor_mul(
                        t[hh * HID : (hh + 1) * HID, :],
                        rl[:],
                        rz2ps[:, h * GPC : (h + 1) * GPC],
                    )
                h2T_f.append(t)

            tap("h2Tf_0", h2T_f[0][:])
            # ---- readout: out = h2 @ out_w + out_b ----
            ops = ptile([GPC, OUT], "outps")
            nc.tensor.matmul(ops[:], h2T_f[0][:], wout0[:], start=True, stop=False)
            nc.tensor.matmul(ops[:], h2T_f[1][:], wout1[:], start=False, stop=False)
            nc.tensor.matmul(
                ops[:], ones_row[:, 0:GPC], woutb[:], start=False, stop=True
            )
            osb = wtile([GPC, OUT], "osb")
            nc.vector.tensor_copy(osb[:], ops[:])
            nc.sync.dma_start(out, osb[:])

    nc.compile()
    return nc


# ----------------------------------------------------------------------------
# Entry point
# ----------------------------------------------------------------------------

def kernel(**inputs) -> np.ndarray:
    from concourse import bass_utils

    obs = np.asarray(inputs["obs"], np.float32)
    pre = _preprocess(obs, inputs["edge_index"])
    in_maps = _per_core_inputs(pre, inputs)

    key = (pre["P_D"], pre["P_S"])
    if key not in _COMPILE_CACHE:
        _COMPILE_CACHE[key] = _build_program(pre["P_D"], pre["P_S"], pre["SC"])
    nc = _COMPILE_CACHE[key]

    res = bass_utils.run_bass_kernel_spmd(nc, in_maps, core_ids=list(range(N_CORES)))
    out = np.concatenate([res.results[c]["out"] for c in range(N_CORES)], axis=0)
    return out.astype(np.float32)
